# revision 41
# baseline (speedup 1.0000x reference)
"""EvolveGCN-O forward on 8 Trainium2 NeuronCores (Bass/Tile).

Math (reference):
    h      = tanh(W_ih @ W_t.flatten() + b_ih + b_hh)   # RNN step, h0 = 0
    W_new  = h.reshape(64, 64)
    xw     = x @ W_new.T
    agg    = segment_sum(xw[src], dst, N)               # 1.6M edges
    out    = relu(agg) @ proj_w.T + proj_b

Key identity: segment_sum(xw[src]) == segment_sum(x[src]) @ W_new.T — the
linear map commutes with the sum, so we aggregate RAW x rows and apply W_new
once per 128-node destination block after aggregation.

Sharding: destinations are sharded across the 8 cores (12544 nodes each, node
space padded to 100352).  Per core:
  - the dst-sorted edge stream is split into 4 src-range groups of 32768 so
    dma_gather's int16 indices can address the x table (stored bf16 at 256B
    row pitch; each gather moves only the 128B payload),
  - one-hot matrices (is_equal against a constant iota, full 128-slot width)
    are built on the DVE; the PE accumulates aggX^T [64 feat x 128 dst] per
    block in PSUM (bf16 x bf16 -> fp32),
  - the RNN matvec is row-sharded (8MB of W_ih per core) + poly-tanh (exact to
    ~1e-9 at these argument scales) + AllGather of h (2KB),
  - per block: W_new^T matmul -> relu -> proj matmul -> +bias (all fp32),
  - outT [128 x 12544] (channel-major) goes out in slabs; host transposes.
"""

import sys
import numpy as np

for _p in ("/opt/trn_rl_repo", "/opt/trn_rl_repo/concourse"):
    if _p not in sys.path:
        sys.path.insert(0, _p)

P = 128          # partitions / dst-block size
D = 64           # feature dim
DOUT = 128       # output feature dim
NCORES = 8
NGRP = 4         # src-range groups (int16 index reach: 32768 rows each)
GRNG = 32768
CB = 49          # chunks per dma_gather call (49*128 = 6272 rows)
HB = 128         # one-hot segment width in dst slots (128 = full psum block;
                 # 64 halves DVE work but costs +60% gather padding on this graph)


def _wrap_idx16(vals):
    """Pack an index vector into dma_gather's SBUF layout:
    idx i -> [i%16, i//16], replicated across the 8 16-partition stripes."""
    n = len(vals)
    assert n % 16 == 0
    a = np.asarray(vals, dtype=np.int16).reshape(n // 16, 16).T  # [16, n/16]
    return np.tile(a, (8, 1))                                    # [128, n/16]


# ----------------------------------------------------------------------------
# Host-side preprocessing
# ----------------------------------------------------------------------------

def prep_schedule(edge_index: np.ndarray, n_nodes: int, n_cores: int = NCORES):
    """Static per-core schedule.

    Edge stream per core: blocks (128 dst nodes) x 4 src-groups; each
    (block, group) segment is dst-sorted and padded to Gg chunks of 128 edges
    (Gg = global max, identical on all cores: one SPMD program).  Group g's
    chunks across blocks form stream g, gathered in CB-chunk batches.

    Returns per-core eidx[g] (wrapped int16), dstloc [128, C] fp32 (block-major
    chunk order, -1 pads), and the schedule constants.
    """
    src = np.asarray(edge_index[0], dtype=np.int64)
    dst = np.asarray(edge_index[1], dtype=np.int64)
    npc = P * int(np.ceil(np.ceil(n_nodes / n_cores) / P))
    nblk = npc // P

    core = dst // npc
    GB = (npc * n_cores) // NGRP     # balanced src-group width (< 32768)
    grp = src // GB
    nhb = npc // HB                                  # half-blocks (64 dst)
    blk = (dst % npc) // HB

    # sort edges by (core, half-block, group, dst)
    order = np.lexsort((dst, grp, blk, core))
    src_s, dst_s, core_s = src[order], dst[order], core[order]
    grp_s, blk_s = grp[order], blk[order]

    seg = ((core_s * nhb + blk_s) * NGRP + grp_s)    # segment id, sorted
    nseg = n_cores * nhb * NGRP
    cnt = np.bincount(seg, minlength=nseg)
    Gg = int(np.ceil(cnt.max() / P))
    NCH = nhb * Gg                                    # chunks per group-stream
    # pad group stream to a CB multiple
    NCHp = int(np.ceil(NCH / CB)) * CB
    C = nhb * NGRP * Gg                               # dstloc columns (real chunks)

    starts = np.zeros(nseg + 1, dtype=np.int64)
    np.cumsum(cnt, out=starts[1:])
    rank = np.arange(len(src_s)) - starts[seg]

    # position inside group-stream: chunk = blk*Gg + rank//128, lane = rank%128
    sch = blk_s * Gg + rank // P
    lane = rank % P

    eidx = np.zeros((n_cores, NGRP, P, NCHp * 8), dtype=np.int16)
    dstloc = np.full((n_cores, P, C), -1.0, dtype=np.float32)
    for c in range(n_cores):
        mc = core_s == c
        for g in range(NGRP):
            m = mc & (grp_s == g)
            flat = np.zeros(NCHp * P, dtype=np.int64)  # pad idx 0 (valid row)
            flat[sch[m] * P + lane[m]] = src_s[m] - g * GB
            assert flat.min() >= 0 and flat.max() < GB and GB <= 32768
            eidx[c, g] = _wrap_idx16(flat)
        # dstloc in block-major consumption order: col = blk*NGRP*Gg + g*Gg + k
        dl = np.full((C, P), -1.0, dtype=np.float32)
        m = mc
        jcol = blk_s[m] * (NGRP * Gg) + grp_s[m] * Gg + (rank[m] // P)
        dl[jcol, lane[m]] = (dst_s[m] % HB).astype(np.float32)
        dstloc[c] = dl.T
    return dict(eidx=eidx, dstloc=dstloc, Gg=Gg, npc=npc, nblk=nblk,
                C=C, NCH=NCH, NCHp=NCHp)


# ----------------------------------------------------------------------------
# Bass program
# ----------------------------------------------------------------------------

def _patch_dma_gather_assert():
    """elem_size_bytes%256 in bass.dma_gather is a transpose-mode restriction
    applied unconditionally; the non-transpose Q7 path is a generic byte mover
    (verified on HW with 128B elements). Relax it so we can gather bf16 rows
    (128B) from a 256B-pitch table."""
    import inspect
    import textwrap
    import concourse.bass as bass
    if getattr(bass.BassGpSimd.dma_gather, "_elem128_patched", False):
        return
    code = inspect.getsource(bass.BassGpSimd.dma_gather)
    code = code.replace("elem_size_bytes > 0 and elem_size_bytes % 256 == 0",
                        "elem_size_bytes > 0 and elem_size_bytes % 128 == 0")
    ns = dict(vars(bass))
    exec(compile(textwrap.dedent(code), "patched_dma_gather", "exec"), ns)
    ns["dma_gather"]._elem128_patched = True
    bass.BassGpSimd.dma_gather = ns["dma_gather"]


def build_program(nblk, Gg, C, NCHp, n_cores=NCORES, timing=False, skip=()):
    import concourse.bass as bass
    import concourse.bacc as bacc
    import concourse.mybir as mybir
    from concourse.tile import TileContext

    _patch_dma_gather_assert()

    f32 = mybir.dt.float32
    bf16 = mybir.dt.bfloat16
    i16 = mybir.dt.int16
    npc = nblk * P
    H = D * D
    HS = H // n_cores
    G = NGRP * Gg                 # chunks per block
    ncall = NCHp // CB            # gather calls per group
    ntab = npc * n_cores

    nc = bacc.Bacc("TRN2", target_bir_lowering=False, debug=False)

    # ---- DRAM I/O ----
    # bf16 x table with 256B row pitch (64 data cols + 64 pad); gathers move
    # only the 128B payload per edge.
    xtab = nc.dram_tensor("xtab", [ntab, 2 * D], bf16, kind="ExternalInput")
    eidx_d = nc.dram_tensor("eidx", [NGRP, P, NCHp * 8], i16, kind="ExternalInput")
    dstloc_d = nc.dram_tensor("dstloc", [P, C], f32, kind="ExternalInput")
    wflatT_d = nc.dram_tensor("wflatT", [P, H // P], f32, kind="ExternalInput")
    wihT_d = nc.dram_tensor("wihT", [H, HS], f32, kind="ExternalInput")
    biases_d = nc.dram_tensor("biases", [1, 2 * HS], f32, kind="ExternalInput")
    projwT_d = nc.dram_tensor("projwT", [D, DOUT], f32, kind="ExternalInput")
    projb_d = nc.dram_tensor("projb", [DOUT, 1], f32, kind="ExternalInput")
    iota_d = nc.dram_tensor("iota", [P, P], f32, kind="ExternalInput")
    ident_d = nc.dram_tensor("ident", [D, D], f32, kind="ExternalInput")
    outT_d = nc.dram_tensor("outT", [DOUT, npc], f32, kind="ExternalOutput")
    hout_d = nc.dram_tensor("h_out", [1, H], f32, kind="ExternalOutput")

    with TileContext(nc) as tc:
        with (
            tc.tile_pool(name="dram", bufs=1, space="DRAM") as dpool,
            tc.tile_pool(name="const", bufs=1) as cpool,
            tc.tile_pool(name="gbuf", bufs=2) as gpool,
            tc.tile_pool(name="oh", bufs=2) as ohpool,
            tc.tile_pool(name="wih", bufs=3) as wihpool,
            tc.tile_pool(name="big", bufs=1) as bigpool,
            tc.tile_pool(name="small", bufs=1) as smpool,
            tc.tile_pool(name="slab", bufs=2) as slabpool,
            tc.tile_pool(name="psA", bufs=3, space="PSUM") as psA,
            tc.tile_pool(name="psH", bufs=1, space="PSUM") as psH,
            tc.tile_pool(name="psB", bufs=2, space="PSUM") as psB,
            tc.tile_pool(name="psC", bufs=2, space="PSUM") as psC,
        ):
            # ---- constants / persistent tiles ----
            dstloc_sb = cpool.tile([P, C], f32)
            iota_sb = cpool.tile([P, P], f32)
            ident_sb = cpool.tile([D, D], f32)
            wflatT_sb = cpool.tile([P, H // P], f32)
            projwT_sb = cpool.tile([D, DOUT], f32)
            projb_sb = cpool.tile([DOUT, 1], f32)
            zeros_sb = cpool.tile([P, P], bf16)
            aggX_sb = bigpool.tile([D, npc], f32, tag="aggX")

            nc.sync.dma_start(out=dstloc_sb[:], in_=dstloc_d[:])
            nc.sync.dma_start(out=iota_sb[:], in_=iota_d[:])
            nc.sync.dma_start(out=ident_sb[:], in_=ident_d[:])
            nc.sync.dma_start(out=wflatT_sb[:], in_=wflatT_d[:])
            nc.sync.dma_start(out=projwT_sb[:], in_=projwT_d[:])
            nc.sync.dma_start(out=projb_sb[:], in_=projb_d[:])
            nc.vector.memset(zeros_sb[:], 0.0)

            # ================= RNN matvec + tanh + allgather =================
            ps_h = psH.tile([1, HS], f32, tag="aux")
            for k in range(H // P):
                wih_sb = wihpool.tile([P, HS], f32)
                nc.sync.dma_start(out=wih_sb[:], in_=wihT_d[k * P:(k + 1) * P, :])
                nc.tensor.matmul(
                    ps_h[:], lhsT=wflatT_sb[:, k:k + 1], rhs=wih_sb[:],
                    start=(k == 0), stop=(k == H // P - 1),
                )
            biases_sb = smpool.tile([1, 2 * HS], f32, tag="hb")
            nc.sync.dma_start(out=biases_sb[:], in_=biases_d[:])
            xh = smpool.tile([1, HS], f32, tag="hx")
            nc.vector.tensor_add(out=xh[:], in0=biases_sb[:, :HS], in1=biases_sb[:, HS:])
            nc.vector.tensor_add(out=xh[:], in0=xh[:], in1=ps_h[:])
            # tanh(x) ~= x * (1 + u*(-1/3 + u*(2/15 + u*(-17/315)))), u = x^2
            u = smpool.tile([1, HS], f32, tag="hu")
            v = smpool.tile([1, HS], f32, tag="hv")
            nc.vector.tensor_mul(out=u[:], in0=xh[:], in1=xh[:])
            nc.vector.tensor_scalar(
                out=v[:], in0=u[:], scalar1=-17.0 / 315.0, scalar2=2.0 / 15.0,
                op0=mybir.AluOpType.mult, op1=mybir.AluOpType.add)
            nc.vector.tensor_mul(out=v[:], in0=v[:], in1=u[:])
            nc.vector.tensor_scalar_add(out=v[:], in0=v[:], scalar1=-1.0 / 3.0)
            nc.vector.tensor_mul(out=v[:], in0=v[:], in1=u[:])
            nc.vector.tensor_scalar_add(out=v[:], in0=v[:], scalar1=1.0)
            h_part = smpool.tile([1, HS], f32, tag="hp")
            nc.vector.tensor_mul(out=h_part[:], in0=v[:], in1=xh[:])

            h_sb = smpool.tile([D, D], f32, tag="hfull")
            hpart_b = dpool.tile([1, HS], f32, tag="hpart")
            hfull_b = dpool.tile([1, H], f32, tag="hfull_d")
            nc.sync.dma_start(out=hpart_b[:], in_=h_part[:])
            if timing:
                # TimelineSim can't model collectives; stand in a local DMA
                nc.sync.dma_start(out=hfull_b[0:1, :HS], in_=hpart_b[:])
            else:
                nc.gpsimd.collective_compute(
                    "AllGather", mybir.AluOpType.bypass,
                    replica_groups=[list(range(n_cores))],
                    ins=[hpart_b.opt()], outs=[hfull_b.opt()],
                )
            nc.sync.dma_start(out=hout_d[:], in_=hfull_b[:])
            nc.sync.dma_start(out=h_sb[:], in_=hfull_b[0, :].rearrange("(a b) -> a b", a=D))
            ps_t = psH.tile([D, D], f32, tag="aux")
            nc.tensor.transpose(out=ps_t[:], in_=h_sb[:], identity=ident_sb[:])
            wnT_sb = smpool.tile([D, D], f32, tag="wnT")
            nc.vector.tensor_copy(out=wnT_sb[:], in_=ps_t[:])

            # ================= phase A: gather + one-hot accumulate ==========
            # NOTE: SWDGE DMA instructions MUST be emitted in strict
            # round-robin queue order (0,1,2,3,0,1,...) — Tile assigns the 8
            # DMASW sem lanes round-robin by emission order and each lane is
            # locked to one queue.
            gtiles = {}
            eidx_sbs = []
            for g in range(NGRP):
                e_sb = cpool.tile([P, NCHp * 8], i16, tag=f"eidx{g}")
                nc.sync.dma_start(out=e_sb[:], in_=eidx_d[g, :, :])
                eidx_sbs.append(e_sb)
            for k in range(ncall):
                for g in range(NGRP):
                    gt = gpool.tile([P, CB, D], bf16, tag=f"g{g}")
                    base = g * (ntab // NGRP)
                    rows = ntab // NGRP
                    if "gather" not in skip:
                        nc.gpsimd.dma_gather(
                            gt[:], xtab[base:base + rows, :D],
                            eidx_sbs[g][:, k * CB * 8:(k + 1) * CB * 8],
                            CB * P, CB * P, D, elem_step=2 * D, queue_num=0,
                            single_packet=False,
                        )
                    gtiles[(g, k)] = gt

            for b in range(nblk):
                ps = psA.tile([D, P], f32)
                nc.tensor.matmul(ps[:], lhsT=zeros_sb[:, :D], rhs=zeros_sb[:],
                                 start=True, stop=False)
                for h in range(P // HB):
                    hb = b * (P // HB) + h    # 64-dst half-block index
                    oh = ohpool.tile([P, G, HB], bf16)
                    j0 = hb * G
                    if "onehot" not in skip:
                        nc.vector.tensor_tensor(
                            out=oh[:],
                            in0=iota_sb[:, None, :HB].to_broadcast([P, G, HB]),
                            in1=dstloc_sb[:, j0:j0 + G, None].to_broadcast([P, G, HB]),
                            op=mybir.AluOpType.is_equal,
                        )
                    for jj in range(G):
                        if "chunkmm" in skip:
                            break
                        g, kk = divmod(jj, Gg)
                        sc = hb * Gg + kk     # group-stream chunk index
                        call, off = divmod(sc, CB)
                        nc.tensor.matmul(
                            ps[:, h * HB:(h + 1) * HB],
                            lhsT=gtiles[(g, call)][:, off, :],
                            rhs=oh[:, jj, :],
                            start=False,
                            stop=(h == P // HB - 1 and jj == G - 1),
                        )
                nc.scalar.activation(out=aggX_sb[:, b * P:(b + 1) * P], in_=ps[:],
                                     func=mybir.ActivationFunctionType.Copy)

            # ================= phase B: W_new^T, relu, proj, bias ============
            SLAB = 14
            slab_t = None
            for b in range(nblk):
                if b % SLAB == 0:
                    slab_t = slabpool.tile([DOUT, SLAB * P], f32)
                so = (b % SLAB) * P
                psb = psB.tile([D, P], f32)
                nc.tensor.matmul(psb[:], lhsT=wnT_sb[:],
                                 rhs=aggX_sb[:, b * P:(b + 1) * P],
                                 start=True, stop=True)
                relu_t = smpool.tile([D, P], f32, tag="relu")
                nc.scalar.activation(out=relu_t[:], in_=psb[:],
                                     func=mybir.ActivationFunctionType.Relu)
                psc = psC.tile([DOUT, P], f32)
                nc.tensor.matmul(psc[:], lhsT=projwT_sb[:], rhs=relu_t[:],
                                 start=True, stop=True)
                nc.scalar.activation(out=slab_t[:, so:so + P], in_=psc[:],
                                     func=mybir.ActivationFunctionType.Identity,
                                     bias=projb_sb[:], scale=1.0)
                if b % SLAB == SLAB - 1 or b == nblk - 1:
                    lo = (b // SLAB) * SLAB * P
                    nc.sync.dma_start(out=outT_d[:, lo:(b + 1) * P],
                                      in_=slab_t[:, :(b + 1) * P - lo])

    nc.compile()
    return nc


# ----------------------------------------------------------------------------
# Entry point
# ----------------------------------------------------------------------------

def make_inputs(inputs, sched):
    """Per-core in_maps from the raw inputs + schedule."""
    x = np.asarray(inputs["x"], dtype=np.float32)
    W_t = np.asarray(inputs["W_t"], dtype=np.float32)
    W_ih = np.asarray(inputs["W_ih"], dtype=np.float32)
    b_ih = np.asarray(inputs["b_ih"], dtype=np.float32)
    b_hh = np.asarray(inputs["b_hh"], dtype=np.float32)
    proj_w = np.asarray(inputs["proj_w"], dtype=np.float32)
    proj_b = np.asarray(inputs["proj_b"], dtype=np.float32)

    n_nodes = x.shape[0]
    H = D * D
    HS = H // NCORES
    npc = sched["npc"]

    import ml_dtypes
    xpad = np.zeros((npc * NCORES, 2 * D), dtype=ml_dtypes.bfloat16)
    xpad[:n_nodes, :D] = x.astype(ml_dtypes.bfloat16)
    wflatT = W_t.reshape(-1).reshape(H // P, P).T.copy()
    projwT = proj_w.T.copy()
    projb = proj_b.reshape(DOUT, 1).copy()
    iota = np.tile(np.arange(P, dtype=np.float32), (P, 1))
    ident = np.eye(D, dtype=np.float32)

    maps = []
    for c in range(NCORES):
        maps.append({
            "xtab": xpad,
            "eidx": sched["eidx"][c],
            "dstloc": sched["dstloc"][c],
            "wflatT": wflatT,
            "wihT": np.ascontiguousarray(W_ih[c * HS:(c + 1) * HS, :].T),
            "biases": np.concatenate([b_ih[c * HS:(c + 1) * HS],
                                      b_hh[c * HS:(c + 1) * HS]]).reshape(1, 2 * HS),
            "projwT": projwT,
            "projb": projb,
            "iota": iota,
            "ident": ident,
        })
    return maps


def kernel(**inputs):
    from concourse.bass_utils import run_bass_kernel_spmd

    x = np.asarray(inputs["x"], dtype=np.float32)
    n_nodes = x.shape[0]

    # safety: poly-tanh valid range
    arg = (np.asarray(inputs["W_ih"], dtype=np.float32) @
           np.asarray(inputs["W_t"], dtype=np.float32).reshape(-1)
           + np.asarray(inputs["b_ih"], dtype=np.float32)
           + np.asarray(inputs["b_hh"], dtype=np.float32))
    assert np.abs(arg).max() < 0.45, np.abs(arg).max()

    sched = prep_schedule(np.asarray(inputs["edge_index"]), n_nodes)
    npc = sched["npc"]

    nc = build_program(sched["nblk"], sched["Gg"], sched["C"], sched["NCHp"])
    in_maps = make_inputs(inputs, sched)

    res = run_bass_kernel_spmd(nc, in_maps, core_ids=list(range(NCORES)))
    kernel.last_results = res

    out = np.empty((n_nodes, DOUT), dtype=np.float32)
    for c in range(NCORES):
        lo = c * npc
        hi = min((c + 1) * npc, n_nodes)
        if hi > lo:
            out[lo:hi] = res.results[c]["outT"][:, :hi - lo].T
    W_new = res.results[0]["h_out"].reshape(D, D).copy()
    return out, W_new


# revision 42
# speedup vs baseline: 1.0254x; 1.0254x over previous
"""EvolveGCN-O forward on 8 Trainium2 NeuronCores (Bass/Tile).

Math (reference):
    h      = tanh(W_ih @ W_t.flatten() + b_ih + b_hh)   # RNN step, h0 = 0
    W_new  = h.reshape(64, 64)
    xw     = x @ W_new.T
    agg    = segment_sum(xw[src], dst, N)               # 1.6M edges
    out    = relu(agg) @ proj_w.T + proj_b

Key identity: segment_sum(xw[src]) == segment_sum(x[src]) @ W_new.T — the
linear map commutes with the sum, so we aggregate RAW x rows and apply W_new
once per 128-node destination block after aggregation.

Sharding: destinations are sharded across the 8 cores (12544 nodes each, node
space padded to 100352).  Per core:
  - the dst-sorted edge stream is split into 4 src-range groups of 32768 so
    dma_gather's int16 indices can address the x table (stored bf16 at 256B
    row pitch; each gather moves only the 128B payload),
  - one-hot matrices (is_equal against a constant iota, full 128-slot width)
    are built on the DVE; the PE accumulates aggX^T [64 feat x 128 dst] per
    block in PSUM (bf16 x bf16 -> fp32),
  - the RNN matvec is row-sharded (8MB of W_ih per core) + poly-tanh (exact to
    ~1e-9 at these argument scales) + AllGather of h (2KB),
  - per block: W_new^T matmul -> relu -> proj matmul -> +bias (all fp32),
  - outT [128 x 12544] (channel-major) goes out in slabs; host transposes.
"""

import sys
import numpy as np

for _p in ("/opt/trn_rl_repo", "/opt/trn_rl_repo/concourse"):
    if _p not in sys.path:
        sys.path.insert(0, _p)

P = 128          # partitions / dst-block size
D = 64           # feature dim
DOUT = 128       # output feature dim
NCORES = 8
NGRP = 4         # src-range groups (int16 index reach: 32768 rows each)
GRNG = 32768
CB = 49          # chunks per dma_gather call (49*128 = 6272 rows)
HB = 128         # one-hot segment width in dst slots (128 = full psum block;
                 # 64 halves DVE work but costs +60% gather padding on this graph)


def _wrap_idx16(vals):
    """Pack an index vector into dma_gather's SBUF layout:
    idx i -> [i%16, i//16], replicated across the 8 16-partition stripes."""
    n = len(vals)
    assert n % 16 == 0
    a = np.asarray(vals, dtype=np.int16).reshape(n // 16, 16).T  # [16, n/16]
    return np.tile(a, (8, 1))                                    # [128, n/16]


# ----------------------------------------------------------------------------
# Host-side preprocessing
# ----------------------------------------------------------------------------

def prep_schedule(edge_index: np.ndarray, n_nodes: int, n_cores: int = NCORES):
    """Static per-core schedule.

    Edge stream per core: blocks (128 dst nodes) x 4 src-groups; each
    (block, group) segment is dst-sorted and padded to Gg chunks of 128 edges
    (Gg = global max, identical on all cores: one SPMD program).  Group g's
    chunks across blocks form stream g, gathered in CB-chunk batches.

    Returns per-core eidx[g] (wrapped int16), dstloc [128, C] fp32 (block-major
    chunk order, -1 pads), and the schedule constants.
    """
    src = np.asarray(edge_index[0], dtype=np.int64)
    dst = np.asarray(edge_index[1], dtype=np.int64)
    npc = P * int(np.ceil(np.ceil(n_nodes / n_cores) / P))
    nblk = npc // P

    core = dst // npc
    GB = (npc * n_cores) // NGRP     # balanced src-group width (< 32768)
    grp = src // GB
    nhb = npc // HB                                  # half-blocks (64 dst)
    blk = (dst % npc) // HB

    # sort edges by (core, half-block, group, dst)
    order = np.lexsort((dst, grp, blk, core))
    src_s, dst_s, core_s = src[order], dst[order], core[order]
    grp_s, blk_s = grp[order], blk[order]

    seg = ((core_s * nhb + blk_s) * NGRP + grp_s)    # segment id, sorted
    nseg = n_cores * nhb * NGRP
    cnt = np.bincount(seg, minlength=nseg)
    Gg = int(np.ceil(cnt.max() / P))
    NCH = nhb * Gg                                    # chunks per group-stream
    # pad group stream to a CB multiple
    NCHp = int(np.ceil(NCH / CB)) * CB
    C = nhb * NGRP * Gg                               # dstloc columns (real chunks)

    starts = np.zeros(nseg + 1, dtype=np.int64)
    np.cumsum(cnt, out=starts[1:])
    rank = np.arange(len(src_s)) - starts[seg]

    # position inside group-stream: chunk = blk*Gg + rank//128, lane = rank%128
    sch = blk_s * Gg + rank // P
    lane = rank % P

    eidx = np.zeros((n_cores, NGRP, P, NCHp * 8), dtype=np.int16)
    dstloc = np.full((n_cores, P, C), -1.0, dtype=np.float32)
    for c in range(n_cores):
        mc = core_s == c
        for g in range(NGRP):
            m = mc & (grp_s == g)
            flat = np.zeros(NCHp * P, dtype=np.int64)  # pad idx 0 (valid row)
            flat[sch[m] * P + lane[m]] = src_s[m] - g * GB
            assert flat.min() >= 0 and flat.max() < GB and GB <= 32768
            eidx[c, g] = _wrap_idx16(flat)
        # dstloc in block-major consumption order: col = blk*NGRP*Gg + g*Gg + k
        dl = np.full((C, P), -1.0, dtype=np.float32)
        m = mc
        jcol = blk_s[m] * (NGRP * Gg) + grp_s[m] * Gg + (rank[m] // P)
        dl[jcol, lane[m]] = (dst_s[m] % HB).astype(np.float32)
        dstloc[c] = dl.T
    return dict(eidx=eidx, dstloc=dstloc, Gg=Gg, npc=npc, nblk=nblk,
                C=C, NCH=NCH, NCHp=NCHp)


# ----------------------------------------------------------------------------
# Bass program
# ----------------------------------------------------------------------------

def _patch_dma_gather_assert():
    """elem_size_bytes%256 in bass.dma_gather is a transpose-mode restriction
    applied unconditionally; the non-transpose Q7 path is a generic byte mover
    (verified on HW with 128B elements). Relax it so we can gather bf16 rows
    (128B) from a 256B-pitch table."""
    import inspect
    import textwrap
    import concourse.bass as bass
    if getattr(bass.BassGpSimd.dma_gather, "_elem128_patched", False):
        return
    code = inspect.getsource(bass.BassGpSimd.dma_gather)
    code = code.replace("elem_size_bytes > 0 and elem_size_bytes % 256 == 0",
                        "elem_size_bytes > 0 and elem_size_bytes % 128 == 0")
    ns = dict(vars(bass))
    exec(compile(textwrap.dedent(code), "patched_dma_gather", "exec"), ns)
    ns["dma_gather"]._elem128_patched = True
    bass.BassGpSimd.dma_gather = ns["dma_gather"]


def build_program(nblk, Gg, C, NCHp, n_cores=NCORES, timing=False, skip=()):
    import concourse.bass as bass
    import concourse.bacc as bacc
    import concourse.mybir as mybir
    from concourse.tile import TileContext

    _patch_dma_gather_assert()

    f32 = mybir.dt.float32
    bf16 = mybir.dt.bfloat16
    i16 = mybir.dt.int16
    npc = nblk * P
    H = D * D
    HS = H // n_cores
    G = NGRP * Gg                 # chunks per block
    ncall = NCHp // CB            # gather calls per group
    ntab = npc * n_cores

    nc = bacc.Bacc("TRN2", target_bir_lowering=False, debug=False)

    # ---- DRAM I/O ----
    # bf16 x table with 256B row pitch (64 data cols + 64 pad); gathers move
    # only the 128B payload per edge.
    xtab = nc.dram_tensor("xtab", [ntab, 2 * D], bf16, kind="ExternalInput")
    eidx_d = nc.dram_tensor("eidx", [NGRP, P, NCHp * 8], i16, kind="ExternalInput")
    dstloc_d = nc.dram_tensor("dstloc", [P, C], f32, kind="ExternalInput")
    wflatT_d = nc.dram_tensor("wflatT", [P, H // P], f32, kind="ExternalInput")
    wihT_d = nc.dram_tensor("wihT", [H, HS], f32, kind="ExternalInput")
    biases_d = nc.dram_tensor("biases", [1, 2 * HS], f32, kind="ExternalInput")
    projwT_d = nc.dram_tensor("projwT", [D, DOUT], f32, kind="ExternalInput")
    projb_d = nc.dram_tensor("projb", [DOUT, 1], f32, kind="ExternalInput")
    iota_d = nc.dram_tensor("iota", [P, P], f32, kind="ExternalInput")
    ident_d = nc.dram_tensor("ident", [D, D], f32, kind="ExternalInput")
    outT_d = nc.dram_tensor("outT", [DOUT, npc], f32, kind="ExternalOutput")
    hout_d = nc.dram_tensor("h_out", [1, H], f32, kind="ExternalOutput")

    with TileContext(nc) as tc:
        with (
            tc.tile_pool(name="dram", bufs=1, space="DRAM") as dpool,
            tc.tile_pool(name="const", bufs=1) as cpool,
            tc.tile_pool(name="gbuf", bufs=2) as gpool,
            tc.tile_pool(name="oh", bufs=4) as ohpool,
            tc.tile_pool(name="wih", bufs=3) as wihpool,
            tc.tile_pool(name="big", bufs=1) as bigpool,
            tc.tile_pool(name="small", bufs=1) as smpool,
            tc.tile_pool(name="slab", bufs=2) as slabpool,
            tc.tile_pool(name="psA", bufs=3, space="PSUM") as psA,
            tc.tile_pool(name="psH", bufs=1, space="PSUM") as psH,
            tc.tile_pool(name="psB", bufs=2, space="PSUM") as psB,
            tc.tile_pool(name="psC", bufs=2, space="PSUM") as psC,
        ):
            # ---- constants / persistent tiles ----
            dstloc_sb = cpool.tile([P, C], f32)
            iota_sb = cpool.tile([P, P], f32)
            ident_sb = cpool.tile([D, D], f32)
            wflatT_sb = cpool.tile([P, H // P], f32)
            projwT_sb = cpool.tile([D, DOUT], f32)
            projb_sb = cpool.tile([DOUT, 1], f32)
            zeros_sb = cpool.tile([P, P], bf16)
            aggX_sb = bigpool.tile([D, npc], f32, tag="aggX")

            nc.sync.dma_start(out=dstloc_sb[:], in_=dstloc_d[:])
            nc.sync.dma_start(out=iota_sb[:], in_=iota_d[:])
            nc.sync.dma_start(out=ident_sb[:], in_=ident_d[:])
            nc.sync.dma_start(out=wflatT_sb[:], in_=wflatT_d[:])
            nc.sync.dma_start(out=projwT_sb[:], in_=projwT_d[:])
            nc.sync.dma_start(out=projb_sb[:], in_=projb_d[:])
            nc.vector.memset(zeros_sb[:], 0.0)

            # ================= RNN matvec + tanh + allgather =================
            ps_h = psH.tile([1, HS], f32, tag="aux")
            for k in range(H // P):
                wih_sb = wihpool.tile([P, HS], f32)
                nc.sync.dma_start(out=wih_sb[:], in_=wihT_d[k * P:(k + 1) * P, :])
                nc.tensor.matmul(
                    ps_h[:], lhsT=wflatT_sb[:, k:k + 1], rhs=wih_sb[:],
                    start=(k == 0), stop=(k == H // P - 1),
                )
            biases_sb = smpool.tile([1, 2 * HS], f32, tag="hb")
            nc.sync.dma_start(out=biases_sb[:], in_=biases_d[:])
            xh = smpool.tile([1, HS], f32, tag="hx")
            nc.vector.tensor_add(out=xh[:], in0=biases_sb[:, :HS], in1=biases_sb[:, HS:])
            nc.vector.tensor_add(out=xh[:], in0=xh[:], in1=ps_h[:])
            # tanh(x) ~= x * (1 + u*(-1/3 + u*(2/15 + u*(-17/315)))), u = x^2
            u = smpool.tile([1, HS], f32, tag="hu")
            v = smpool.tile([1, HS], f32, tag="hv")
            nc.vector.tensor_mul(out=u[:], in0=xh[:], in1=xh[:])
            nc.vector.tensor_scalar(
                out=v[:], in0=u[:], scalar1=-17.0 / 315.0, scalar2=2.0 / 15.0,
                op0=mybir.AluOpType.mult, op1=mybir.AluOpType.add)
            nc.vector.tensor_mul(out=v[:], in0=v[:], in1=u[:])
            nc.vector.tensor_scalar_add(out=v[:], in0=v[:], scalar1=-1.0 / 3.0)
            nc.vector.tensor_mul(out=v[:], in0=v[:], in1=u[:])
            nc.vector.tensor_scalar_add(out=v[:], in0=v[:], scalar1=1.0)
            h_part = smpool.tile([1, HS], f32, tag="hp")
            nc.vector.tensor_mul(out=h_part[:], in0=v[:], in1=xh[:])

            h_sb = smpool.tile([D, D], f32, tag="hfull")
            hpart_b = dpool.tile([1, HS], f32, tag="hpart")
            hfull_b = dpool.tile([1, H], f32, tag="hfull_d")
            nc.sync.dma_start(out=hpart_b[:], in_=h_part[:])
            if timing:
                # TimelineSim can't model collectives; stand in a local DMA
                nc.sync.dma_start(out=hfull_b[0:1, :HS], in_=hpart_b[:])
            else:
                nc.gpsimd.collective_compute(
                    "AllGather", mybir.AluOpType.bypass,
                    replica_groups=[list(range(n_cores))],
                    ins=[hpart_b.opt()], outs=[hfull_b.opt()],
                )
            nc.sync.dma_start(out=hout_d[:], in_=hfull_b[:])
            nc.sync.dma_start(out=h_sb[:], in_=hfull_b[0, :].rearrange("(a b) -> a b", a=D))
            ps_t = psH.tile([D, D], f32, tag="aux")
            nc.tensor.transpose(out=ps_t[:], in_=h_sb[:], identity=ident_sb[:])
            wnT_sb = smpool.tile([D, D], f32, tag="wnT")
            nc.vector.tensor_copy(out=wnT_sb[:], in_=ps_t[:])

            # ================= phase A: gather + one-hot accumulate ==========
            # NOTE: SWDGE DMA instructions MUST be emitted in strict
            # round-robin queue order (0,1,2,3,0,1,...) — Tile assigns the 8
            # DMASW sem lanes round-robin by emission order and each lane is
            # locked to one queue.
            gtiles = {}
            eidx_sbs = []
            for g in range(NGRP):
                e_sb = cpool.tile([P, NCHp * 8], i16, tag=f"eidx{g}")
                nc.sync.dma_start(out=e_sb[:], in_=eidx_d[g, :, :])
                eidx_sbs.append(e_sb)
            for k in range(ncall):
                for g in range(NGRP):
                    gt = gpool.tile([P, CB, D], bf16, tag=f"g{g}")
                    base = g * (ntab // NGRP)
                    rows = ntab // NGRP
                    if "gather" not in skip:
                        nc.gpsimd.dma_gather(
                            gt[:], xtab[base:base + rows, :D],
                            eidx_sbs[g][:, k * CB * 8:(k + 1) * CB * 8],
                            CB * P, CB * P, D, elem_step=2 * D, queue_num=0,
                            single_packet=False,
                        )
                    gtiles[(g, k)] = gt

            for b in range(nblk):
                ps = psA.tile([D, P], f32)
                nc.tensor.matmul(ps[:], lhsT=zeros_sb[:, :D], rhs=zeros_sb[:],
                                 start=True, stop=False)
                for h in range(P // HB):
                    hb = b * (P // HB) + h    # 64-dst half-block index
                    oh = ohpool.tile([P, G, HB], bf16)
                    j0 = hb * G
                    if "onehot" not in skip:
                        nc.vector.tensor_tensor(
                            out=oh[:],
                            in0=iota_sb[:, None, :HB].to_broadcast([P, G, HB]),
                            in1=dstloc_sb[:, j0:j0 + G, None].to_broadcast([P, G, HB]),
                            op=mybir.AluOpType.is_equal,
                        )
                    for jj in range(G):
                        if "chunkmm" in skip:
                            break
                        g, kk = divmod(jj, Gg)
                        sc = hb * Gg + kk     # group-stream chunk index
                        call, off = divmod(sc, CB)
                        nc.tensor.matmul(
                            ps[:, h * HB:(h + 1) * HB],
                            lhsT=gtiles[(g, call)][:, off, :],
                            rhs=oh[:, jj, :],
                            start=False,
                            stop=(h == P // HB - 1 and jj == G - 1),
                        )
                nc.scalar.activation(out=aggX_sb[:, b * P:(b + 1) * P], in_=ps[:],
                                     func=mybir.ActivationFunctionType.Copy)

            # ================= phase B: W_new^T, relu, proj, bias ============
            SLAB = 14
            slab_t = None
            for b in range(nblk):
                if b % SLAB == 0:
                    slab_t = slabpool.tile([DOUT, SLAB * P], f32)
                so = (b % SLAB) * P
                psb = psB.tile([D, P], f32)
                nc.tensor.matmul(psb[:], lhsT=wnT_sb[:],
                                 rhs=aggX_sb[:, b * P:(b + 1) * P],
                                 start=True, stop=True)
                relu_t = smpool.tile([D, P], f32, tag="relu")
                nc.scalar.activation(out=relu_t[:], in_=psb[:],
                                     func=mybir.ActivationFunctionType.Relu)
                psc = psC.tile([DOUT, P], f32)
                nc.tensor.matmul(psc[:], lhsT=projwT_sb[:], rhs=relu_t[:],
                                 start=True, stop=True)
                nc.scalar.activation(out=slab_t[:, so:so + P], in_=psc[:],
                                     func=mybir.ActivationFunctionType.Identity,
                                     bias=projb_sb[:], scale=1.0)
                if b % SLAB == SLAB - 1 or b == nblk - 1:
                    lo = (b // SLAB) * SLAB * P
                    nc.sync.dma_start(out=outT_d[:, lo:(b + 1) * P],
                                      in_=slab_t[:, :(b + 1) * P - lo])

    nc.compile()
    return nc


# ----------------------------------------------------------------------------
# Entry point
# ----------------------------------------------------------------------------

def make_inputs(inputs, sched):
    """Per-core in_maps from the raw inputs + schedule."""
    x = np.asarray(inputs["x"], dtype=np.float32)
    W_t = np.asarray(inputs["W_t"], dtype=np.float32)
    W_ih = np.asarray(inputs["W_ih"], dtype=np.float32)
    b_ih = np.asarray(inputs["b_ih"], dtype=np.float32)
    b_hh = np.asarray(inputs["b_hh"], dtype=np.float32)
    proj_w = np.asarray(inputs["proj_w"], dtype=np.float32)
    proj_b = np.asarray(inputs["proj_b"], dtype=np.float32)

    n_nodes = x.shape[0]
    H = D * D
    HS = H // NCORES
    npc = sched["npc"]

    import ml_dtypes
    xpad = np.zeros((npc * NCORES, 2 * D), dtype=ml_dtypes.bfloat16)
    xpad[:n_nodes, :D] = x.astype(ml_dtypes.bfloat16)
    wflatT = W_t.reshape(-1).reshape(H // P, P).T.copy()
    projwT = proj_w.T.copy()
    projb = proj_b.reshape(DOUT, 1).copy()
    iota = np.tile(np.arange(P, dtype=np.float32), (P, 1))
    ident = np.eye(D, dtype=np.float32)

    maps = []
    for c in range(NCORES):
        maps.append({
            "xtab": xpad,
            "eidx": sched["eidx"][c],
            "dstloc": sched["dstloc"][c],
            "wflatT": wflatT,
            "wihT": np.ascontiguousarray(W_ih[c * HS:(c + 1) * HS, :].T),
            "biases": np.concatenate([b_ih[c * HS:(c + 1) * HS],
                                      b_hh[c * HS:(c + 1) * HS]]).reshape(1, 2 * HS),
            "projwT": projwT,
            "projb": projb,
            "iota": iota,
            "ident": ident,
        })
    return maps


def kernel(**inputs):
    from concourse.bass_utils import run_bass_kernel_spmd

    x = np.asarray(inputs["x"], dtype=np.float32)
    n_nodes = x.shape[0]

    # safety: poly-tanh valid range
    arg = (np.asarray(inputs["W_ih"], dtype=np.float32) @
           np.asarray(inputs["W_t"], dtype=np.float32).reshape(-1)
           + np.asarray(inputs["b_ih"], dtype=np.float32)
           + np.asarray(inputs["b_hh"], dtype=np.float32))
    assert np.abs(arg).max() < 0.45, np.abs(arg).max()

    sched = prep_schedule(np.asarray(inputs["edge_index"]), n_nodes)
    npc = sched["npc"]

    nc = build_program(sched["nblk"], sched["Gg"], sched["C"], sched["NCHp"])
    in_maps = make_inputs(inputs, sched)

    res = run_bass_kernel_spmd(nc, in_maps, core_ids=list(range(NCORES)))
    kernel.last_results = res

    out = np.empty((n_nodes, DOUT), dtype=np.float32)
    for c in range(NCORES):
        lo = c * npc
        hi = min((c + 1) * npc, n_nodes)
        if hi > lo:
            out[lo:hi] = res.results[c]["outT"][:, :hi - lo].T
    W_new = res.results[0]["h_out"].reshape(D, D).copy()
    return out, W_new
